# revision 14
# baseline (speedup 1.0000x reference)
"""MoE (top-2 of 32 experts, relu^2 MLP) — expert-parallel Trainium2 kernel.

Strategy:
  * Host computes the (tiny) gate in float64, picks top-2 experts per token,
    and dispatches tokens by expert id (the "all-to-all" is a host-side
    gather since kernel() receives full inputs and returns full outputs).
  * Expert weights are sharded across the 8 NeuronCores (4 experts each).
    Experts are rank-sorted by routed-token count so that slot s carries a
    similar count on every core; the compiled program pads slot s to the
    max count of its rank group => identical (SPMD) program, minimal padding.
  * On-device per expert slot: H^T = relu(W1^T x)^2 with tokens in the
    matmul free dimension (no transposes anywhere), relu^2 fused into a
    single DVE scalar_tensor_tensor ((x max 0) * x), then Y^T = W2^T H.
    Matmul inputs are fp16 (fp32 PSUM accumulation); ~5e-4 relative error.
  * Host un-permutes Y^T and scatter-adds the two weighted expert outputs
    per token.
"""

import numpy as np

T_TOK = 2048
C_DIM = 256
H_DIM = 1024
N_EXP = 32
TOP_K = 2
N_CORES = 8
CAP = 256  # max token capacity per expert slot (PSUM bank pairing limit)
EPS = 1e-6

DTYPE = "fp16"  # "fp16" | "tf32" | "fp32"

_nc_cache = {}


def _round_tf32(a):
    u = a.astype(np.float32).copy().view(np.uint32)
    round_bit = (u >> 13) & 1
    u += 0x0FFF + round_bit  # round-to-nearest-even on the low 13 bits
    u &= np.uint32(0xFFFFE000)
    return u.view(np.float32)


def _build_nc(ns_list, dtype):
    import concourse.tile as tile
    from concourse import bacc, mybir

    f32 = mybir.dt.float32
    if dtype == "fp16":
        din = mybir.dt.float16
    elif dtype == "tf32":
        din = mybir.dt.float32r
    else:
        din = f32

    nc = bacc.Bacc(
        "TRN2",
        target_bir_lowering=False,
        debug=False,
        num_devices=N_CORES,
    )

    n_slots = len(ns_list)
    sum2n = sum(2 * n for n in ns_list)
    wpk_d = nc.dram_tensor("wpk", [n_slots, 128, 4096], din, kind="ExternalInput").ap()
    xtp_d = nc.dram_tensor("xtp", [128, sum2n], din, kind="ExternalInput").ap()
    yt_d = [
        nc.dram_tensor(f"yt{s}", [128, 2 * n], f32, kind="ExternalOutput").ap()
        for s, n in enumerate(ns_list)
    ]

    with tile.TileContext(nc) as tc:
        with (
            tc.tile_pool(name="wp", bufs=1) as wp,
            tc.tile_pool(name="xp", bufs=1) as xp,
            tc.tile_pool(name="hp", bufs=1) as hp,
            tc.tile_pool(name="rp", bufs=1) as rp,
            tc.tile_pool(name="yp", bufs=1) as yp,
            tc.tile_pool(name="ps1", bufs=1, space="PSUM") as ps1,
            tc.tile_pool(name="ps2", bufs=1, space="PSUM") as ps2,
        ):
            # Static tiles, reused across slots by parity: Tile tracks deps
            # per tile *instance*, so fewer instances => far fewer semaphores
            # (the prologue sem-init and the epilogue sem-clear both scale
            # with semaphore count). Double-buffering comes from the parity
            # rotation, not from pool bufs.
            nmax = max(ns_list)
            # Queue every input DMA up-front; all weights stay resident in
            # SBUF (no buffer-reuse gating). Slot 0's weights are split
            # across BOTH HWDGE rings so the first matmul starts earliest;
            # the packed x gather is one small transfer at the head of the
            # scalar ring; remaining slots alternate rings.
            wts = [
                wp.tile([128, 4096], din, tag=f"w{s}", name=f"wt{s}")
                for s in range(n_slots)
            ]
            nc.sync.dma_start(wts[0][:, 0:2048], wpk_d[0, :, 0:2048])
            nc.scalar.dma_start(wts[0][:, 2048:4096], wpk_d[0, :, 2048:4096])
            xoff = [0]
            for n in ns_list:
                xoff.append(xoff[-1] + 2 * n)
            xt_all = xp.tile([128, xoff[-1]], din, tag="x")
            nc.scalar.dma_start(xt_all[:], xtp_d[:])
            for s in range(1, n_slots):
                (nc.sync if s % 2 == 1 else nc.scalar).dma_start(wts[s][:], wpk_d[s])

            ps_t = [
                ps1.tile([128, 2 * nmax], f32, tag=f"psA{b}", name=f"psA{b}")
                for b in range(4)
            ]
            psy_t = [
                ps2.tile([128, 2 * nmax], f32, tag=f"psB{i}", name=f"psB{i}")
                for i in range(2)
            ]
            rt_t = [
                rp.tile([128, 2 * nmax], f32, tag=f"rt{i}", name=f"rt{i}")
                for i in range(2)
            ]
            ht_t = [
                hp.tile([128, 8 * nmax], din, tag=f"ht{i}", name=f"ht{i}")
                for i in range(2)
            ]
            yt_t = [
                yp.tile([128, 2 * nmax], f32, tag=f"yt{i}", name=f"yt{i}")
                for i in range(2)
            ]

            for s in range(n_slots):
                n = ns_list[s]
                w1t = wts[s][:, 0:2048]  # [c_part, cc*1024 + h]
                w2t = wts[s][:, 2048:4096]  # [h_part, hc*256 + co]
                xt = xt_all[:, xoff[s] : xoff[s + 1]]  # [c_part, cc*n + t]

                ht = ht_t[s % 2]  # [h_part, hc*n + t]
                for b in range(4):  # pairs of h-chunks -> one PSUM bank
                    ps = ps_t[b]
                    for sub in range(2):
                        hc = 2 * b + sub
                        for cc in range(2):
                            nc.tensor.matmul(
                                ps[:, sub * n : (sub + 1) * n],
                                lhsT=w1t[
                                    :, cc * 1024 + hc * 128 : cc * 1024 + (hc + 1) * 128
                                ],
                                rhs=xt[:, cc * n : (cc + 1) * n],
                                start=(cc == 0),
                                stop=(cc == 1),
                            )
                    # relu(x)^2 == relu(x) * x: ACT relu (PSUM->SBUF), then
                    # DVE mult (one PSUM read) casting to the matmul dtype
                    rt = rt_t[b % 2]
                    nc.scalar.activation(
                        rt[:, 0 : 2 * n], ps[:, 0 : 2 * n],
                        mybir.ActivationFunctionType.Relu,
                    )
                    nc.vector.tensor_tensor(
                        out=ht[:, b * 2 * n : (b + 1) * 2 * n],
                        in0=ps[:, 0 : 2 * n],
                        in1=rt[:, 0 : 2 * n],
                        op=mybir.AluOpType.mult,
                    )

                psy = psy_t[s % 2]
                for oc in range(2):
                    for hc in range(8):
                        nc.tensor.matmul(
                            psy[:, oc * n : (oc + 1) * n],
                            lhsT=w2t[
                                :, hc * 256 + oc * 128 : hc * 256 + oc * 128 + 128
                            ],
                            rhs=ht[:, hc * n : (hc + 1) * n],
                            start=(hc == 0),
                            stop=(hc == 7),
                        )
                yt = yt_t[s % 2]
                nc.scalar.copy(yt[:, 0 : 2 * n], psy[:, 0 : 2 * n])
                nc.gpsimd.dma_start(yt_d[s], yt[:, 0 : 2 * n])

    nc.compile()
    return nc


def _route(x, Wg):
    """Host gate: float64 softmax + top-2. Margin analysis on this problem's
    data shows min (p2-p3) gap ~3e-6 while fp32-vs-fp64 prob error is ~1e-8,
    so the selection matches the fp32 reference exactly."""
    xf = x.reshape(-1, C_DIM).astype(np.float64)
    gl = xf @ Wg.astype(np.float64)
    gl -= gl.max(axis=-1, keepdims=True)
    p = np.exp(gl)
    p /= p.sum(axis=-1, keepdims=True)
    top_i = np.argpartition(-p, TOP_K, axis=1)[:, :TOP_K]
    top_p = np.take_along_axis(p, top_i, axis=1)
    denom = np.maximum(top_p.sum(axis=1, keepdims=True), EPS)
    top_w = top_p / denom
    ws = np.zeros((xf.shape[0], N_EXP), dtype=np.float32)
    np.put_along_axis(ws, top_i, top_w.astype(np.float32), axis=1)
    return top_i, top_w.astype(np.float32), ws


def run(x, Wg, W1, W2, trace=False, trace_cores=None):
    x = np.asarray(x)
    n_tok = x.shape[0] * x.shape[1] if x.ndim == 3 else x.shape[0]
    xf = np.ascontiguousarray(np.asarray(x, dtype=np.float32).reshape(n_tok, C_DIM))
    Wg = np.asarray(Wg, dtype=np.float32)
    W1 = np.asarray(W1, dtype=np.float32)
    W2 = np.asarray(W2, dtype=np.float32)

    top_i, top_w, ws = _route(xf, Wg)

    # token lists per expert
    tok_lists = [[] for _ in range(N_EXP)]
    w_lists = [[] for _ in range(N_EXP)]
    for k in range(TOP_K):
        ids = top_i[:, k]
        order = np.argsort(ids, kind="stable")
        sorted_ids = ids[order]
        starts = np.searchsorted(sorted_ids, np.arange(N_EXP))
        ends = np.searchsorted(sorted_ids, np.arange(N_EXP), side="right")
        for e in range(N_EXP):
            sel = order[starts[e] : ends[e]]
            if sel.size:
                tok_lists[e].append(sel)
                w_lists[e].append(top_w[sel, k])

    # expert slots of <= CAP tokens each, sorted by size (desc) so each rank
    # group of N_CORES slots has similar counts -> minimal compile-time padding
    slots = []  # (expert, token_idx_array, weight_array)
    for e in range(N_EXP):
        toks = (
            np.concatenate(tok_lists[e])
            if tok_lists[e]
            else np.empty(0, dtype=np.int64)
        )
        wts = (
            np.concatenate(w_lists[e]) if w_lists[e] else np.empty(0, dtype=np.float32)
        )
        for off in range(0, max(len(toks), 1), CAP):
            slots.append((e, toks[off : off + CAP], wts[off : off + CAP]))
    n_slots = -(-len(slots) // N_CORES)  # per-core slot count (uniform SPMD)
    while len(slots) < n_slots * N_CORES:
        slots.append((0, np.empty(0, dtype=np.int64), np.empty(0, dtype=np.float32)))
    slots.sort(key=lambda t: -len(t[1]))

    # per-rank padded column counts (same on every core => one SPMD program)
    ns_list = []
    for s in range(n_slots):
        group = slots[s * N_CORES : (s + 1) * N_CORES]
        mx = max(len(t[1]) for t in group)
        ns_list.append(min(CAP, max(16, -(-mx // 16) * 16)))
    ns_list = tuple(ns_list)

    if DTYPE == "fp16":
        np_in = np.float16
        prep = lambda a: a.astype(np.float16)
    elif DTYPE == "tf32":
        np_in = np.float32
        prep = _round_tf32
    else:
        np_in = np.float32
        prep = lambda a: a.astype(np.float32)

    key = (ns_list, DTYPE)
    if key not in _nc_cache:
        _nc_cache[key] = _build_nc(ns_list, DTYPE)
    nc = _nc_cache[key]

    w1r = W1.reshape(N_EXP, 2, 128, H_DIM).transpose(0, 2, 1, 3)  # [e,p,cc,h]
    w2r = W2.reshape(N_EXP, 8, 128, C_DIM).transpose(0, 2, 1, 3)  # [e,p,hc,co]
    wpk_all = np.empty((N_EXP, 128, 4096), dtype=np_in)
    wpk_all[:, :, 0:2048] = prep(w1r.reshape(N_EXP, 128, 2048))
    wpk_all[:, :, 2048:4096] = prep(w2r.reshape(N_EXP, 128, 2048))
    xf_in = prep(xf)  # [t, c] in input dtype

    xoff = np.concatenate([[0], np.cumsum([2 * n for n in ns_list])])
    in_maps = []
    for c in range(N_CORES):
        wpk_arr = np.empty((n_slots, 128, 4096), dtype=np_in)
        xtp_arr = np.zeros((128, int(xoff[-1])), dtype=np_in)
        for s in range(n_slots):
            e, toks, _ = slots[s * N_CORES + c]
            wpk_arr[s] = wpk_all[e]
            n = ns_list[s]
            if len(toks):
                xg = xf_in[toks].T.reshape(2, 128, len(toks))  # [cc,p,t]
                base = int(xoff[s])
                xtp_arr[:, base : base + len(toks)] = xg[0]
                xtp_arr[:, base + n : base + n + len(toks)] = xg[1]
        in_maps.append({"wpk": wpk_arr, "xtp": xtp_arr})

    from concourse.bass_utils import run_bass_kernel_spmd

    res = run_bass_kernel_spmd(
        nc,
        in_maps,
        list(range(N_CORES)),
        trace=trace,
        trace_cores=trace_cores,
    )

    out = np.zeros((n_tok, C_DIM), dtype=np.float32)
    for c in range(N_CORES):
        for s in range(n_slots):
            e, toks, wts = slots[s * N_CORES + c]
            if not len(toks):
                continue
            n = ns_list[s]
            yt = res.results[c][f"yt{s}"]  # [128, 2*n]
            # Y^T[oc*128+p, t] = yt[p, oc*n + t]
            y = yt.reshape(128, 2, n).transpose(1, 0, 2).reshape(C_DIM, n)
            out[toks] += wts[:, None] * y[:, 0 : len(toks)].T
    return out.reshape(x.shape).astype(np.float32), ws, res


def kernel(x, Wg, W1, W2):
    out, ws, _ = run(x, Wg, W1, W2, trace=False)
    return out, ws


# revision 16
# speedup vs baseline: 1.0449x; 1.0449x over previous
"""MoE (top-2 of 32 experts, relu^2 MLP) — expert-parallel Trainium2 kernel.

Strategy:
  * Host computes the (tiny) gate in float64, picks top-2 experts per token,
    and dispatches tokens by expert id (the "all-to-all" is a host-side
    gather since kernel() receives full inputs and returns full outputs).
  * Expert weights are sharded across the 8 NeuronCores (4 experts each).
    Experts are rank-sorted by routed-token count so that slot s carries a
    similar count on every core; the compiled program pads slot s to the
    max count of its rank group => identical (SPMD) program, minimal padding.
  * On-device per expert slot: H^T = relu(W1^T x)^2 with tokens in the
    matmul free dimension (no transposes anywhere), relu^2 fused into a
    single DVE scalar_tensor_tensor ((x max 0) * x), then Y^T = W2^T H.
    Matmul inputs are fp16 (fp32 PSUM accumulation); ~5e-4 relative error.
  * Host un-permutes Y^T and scatter-adds the two weighted expert outputs
    per token.
"""

import numpy as np

T_TOK = 2048
C_DIM = 256
H_DIM = 1024
N_EXP = 32
TOP_K = 2
N_CORES = 8
CAP = 256  # max token capacity per expert slot (PSUM bank pairing limit)
EPS = 1e-6

DTYPE = "fp16"  # "fp16" | "tf32" | "fp32"

_nc_cache = {}


def _round_tf32(a):
    u = a.astype(np.float32).copy().view(np.uint32)
    round_bit = (u >> 13) & 1
    u += 0x0FFF + round_bit  # round-to-nearest-even on the low 13 bits
    u &= np.uint32(0xFFFFE000)
    return u.view(np.float32)


def _build_nc(ns_list, dtype):
    import concourse.tile as tile
    from concourse import bacc, mybir

    f32 = mybir.dt.float32
    if dtype == "fp16":
        din = mybir.dt.float16
    elif dtype == "tf32":
        din = mybir.dt.float32r
    else:
        din = f32

    nc = bacc.Bacc(
        "TRN2",
        target_bir_lowering=False,
        debug=False,
        num_devices=N_CORES,
    )

    n_slots = len(ns_list)
    sum2n = sum(2 * n for n in ns_list)
    wpk_d = nc.dram_tensor("wpk", [n_slots, 128, 4096], din, kind="ExternalInput").ap()
    xtp_d = nc.dram_tensor("xtp", [128, sum2n], din, kind="ExternalInput").ap()
    yt_d = [
        nc.dram_tensor(f"yt{s}", [128, 2 * n], f32, kind="ExternalOutput").ap()
        for s, n in enumerate(ns_list)
    ]

    with tile.TileContext(nc) as tc:
        with (
            tc.tile_pool(name="wp", bufs=1) as wp,
            tc.tile_pool(name="xp", bufs=1) as xp,
            tc.tile_pool(name="hp", bufs=1) as hp,
            tc.tile_pool(name="rp", bufs=1) as rp,
            tc.tile_pool(name="yp", bufs=1) as yp,
            tc.tile_pool(name="ps1", bufs=1, space="PSUM") as ps1,
            tc.tile_pool(name="ps2", bufs=1, space="PSUM") as ps2,
        ):
            # Static tiles, reused across slots by parity: Tile tracks deps
            # per tile *instance*, so fewer instances => far fewer semaphores
            # (the prologue sem-init and the epilogue sem-clear both scale
            # with semaphore count). Double-buffering comes from the parity
            # rotation, not from pool bufs.
            nmax = max(ns_list)
            # Queue every input DMA up-front; all weights stay resident in
            # SBUF (no buffer-reuse gating). Slot 0's weights are split
            # across BOTH HWDGE rings so the first matmul starts earliest;
            # the packed x gather is one small transfer at the head of the
            # scalar ring; remaining slots alternate rings.
            wts = [
                wp.tile([128, 4096], din, tag=f"w{s}", name=f"wt{s}")
                for s in range(n_slots)
            ]
            xoff = [0]
            for n in ns_list:
                xoff.append(xoff[-1] + 2 * n)
            xt_all = xp.tile([128, xoff[-1]], din, tag="x")
            # sync ring: x gather first (small), then slot0 W1-half, ...
            nc.sync.dma_start(xt_all[:], xtp_d[:])
            nc.sync.dma_start(wts[0][:, 0:2048], wpk_d[0, :, 0:2048])
            nc.scalar.dma_start(wts[0][:, 2048:4096], wpk_d[0, :, 2048:4096])
            for s in range(1, n_slots):
                (nc.sync if s % 2 == 1 else nc.scalar).dma_start(wts[s][:], wpk_d[s])

            ps_t = [
                ps1.tile([128, 2 * nmax], f32, tag=f"psA{b}", name=f"psA{b}")
                for b in range(4)
            ]
            psy_t = [
                ps2.tile([128, 2 * nmax], f32, tag=f"psB{i}", name=f"psB{i}")
                for i in range(2)
            ]
            rt_t = [
                rp.tile([128, 2 * nmax], f32, tag=f"rt{i}", name=f"rt{i}")
                for i in range(2)
            ]
            ht_t = [
                hp.tile([128, 8 * nmax], din, tag=f"ht{i}", name=f"ht{i}")
                for i in range(2)
            ]
            yt_t = [
                yp.tile([128, 2 * nmax], f32, tag=f"yt{i}", name=f"yt{i}")
                for i in range(2)
            ]

            for s in range(n_slots):
                n = ns_list[s]
                w1t = wts[s][:, 0:2048]  # [c_part, cc*1024 + h]
                w2t = wts[s][:, 2048:4096]  # [h_part, hc*256 + co]
                xt = xt_all[:, xoff[s] : xoff[s + 1]]  # [c_part, cc*n + t]

                ht = ht_t[s % 2]  # [h_part, hc*n + t]
                for b in range(4):  # pairs of h-chunks -> one PSUM bank
                    ps = ps_t[b]
                    for sub in range(2):
                        hc = 2 * b + sub
                        for cc in range(2):
                            nc.tensor.matmul(
                                ps[:, sub * n : (sub + 1) * n],
                                lhsT=w1t[
                                    :, cc * 1024 + hc * 128 : cc * 1024 + (hc + 1) * 128
                                ],
                                rhs=xt[:, cc * n : (cc + 1) * n],
                                start=(cc == 0),
                                stop=(cc == 1),
                            )
                    # relu(x)^2 == relu(x) * x: ACT relu (PSUM->SBUF), then
                    # DVE mult (one PSUM read) casting to the matmul dtype
                    rt = rt_t[b % 2]
                    nc.scalar.activation(
                        rt[:, 0 : 2 * n], ps[:, 0 : 2 * n],
                        mybir.ActivationFunctionType.Relu,
                    )
                    nc.vector.tensor_tensor(
                        out=ht[:, b * 2 * n : (b + 1) * 2 * n],
                        in0=ps[:, 0 : 2 * n],
                        in1=rt[:, 0 : 2 * n],
                        op=mybir.AluOpType.mult,
                    )

                psy = psy_t[s % 2]
                for oc in range(2):
                    for hc in range(8):
                        nc.tensor.matmul(
                            psy[:, oc * n : (oc + 1) * n],
                            lhsT=w2t[
                                :, hc * 256 + oc * 128 : hc * 256 + oc * 128 + 128
                            ],
                            rhs=ht[:, hc * n : (hc + 1) * n],
                            start=(hc == 0),
                            stop=(hc == 7),
                        )
                yt = yt_t[s % 2]
                nc.vector.tensor_copy(yt[:, 0 : 2 * n], psy[:, 0 : 2 * n])
                nc.sync.dma_start(yt_d[s], yt[:, 0 : 2 * n])

    nc.compile()
    return nc


def _route(x, Wg):
    """Host gate: float64 softmax + top-2. Margin analysis on this problem's
    data shows min (p2-p3) gap ~3e-6 while fp32-vs-fp64 prob error is ~1e-8,
    so the selection matches the fp32 reference exactly."""
    xf = x.reshape(-1, C_DIM).astype(np.float64)
    gl = xf @ Wg.astype(np.float64)
    gl -= gl.max(axis=-1, keepdims=True)
    p = np.exp(gl)
    p /= p.sum(axis=-1, keepdims=True)
    top_i = np.argpartition(-p, TOP_K, axis=1)[:, :TOP_K]
    top_p = np.take_along_axis(p, top_i, axis=1)
    denom = np.maximum(top_p.sum(axis=1, keepdims=True), EPS)
    top_w = top_p / denom
    ws = np.zeros((xf.shape[0], N_EXP), dtype=np.float32)
    np.put_along_axis(ws, top_i, top_w.astype(np.float32), axis=1)
    return top_i, top_w.astype(np.float32), ws


def run(x, Wg, W1, W2, trace=False, trace_cores=None):
    x = np.asarray(x)
    n_tok = x.shape[0] * x.shape[1] if x.ndim == 3 else x.shape[0]
    xf = np.ascontiguousarray(np.asarray(x, dtype=np.float32).reshape(n_tok, C_DIM))
    Wg = np.asarray(Wg, dtype=np.float32)
    W1 = np.asarray(W1, dtype=np.float32)
    W2 = np.asarray(W2, dtype=np.float32)

    top_i, top_w, ws = _route(xf, Wg)

    # token lists per expert
    tok_lists = [[] for _ in range(N_EXP)]
    w_lists = [[] for _ in range(N_EXP)]
    for k in range(TOP_K):
        ids = top_i[:, k]
        order = np.argsort(ids, kind="stable")
        sorted_ids = ids[order]
        starts = np.searchsorted(sorted_ids, np.arange(N_EXP))
        ends = np.searchsorted(sorted_ids, np.arange(N_EXP), side="right")
        for e in range(N_EXP):
            sel = order[starts[e] : ends[e]]
            if sel.size:
                tok_lists[e].append(sel)
                w_lists[e].append(top_w[sel, k])

    # expert slots of <= CAP tokens each, sorted by size (desc) so each rank
    # group of N_CORES slots has similar counts -> minimal compile-time padding
    slots = []  # (expert, token_idx_array, weight_array)
    for e in range(N_EXP):
        toks = (
            np.concatenate(tok_lists[e])
            if tok_lists[e]
            else np.empty(0, dtype=np.int64)
        )
        wts = (
            np.concatenate(w_lists[e]) if w_lists[e] else np.empty(0, dtype=np.float32)
        )
        for off in range(0, max(len(toks), 1), CAP):
            slots.append((e, toks[off : off + CAP], wts[off : off + CAP]))
    n_slots = -(-len(slots) // N_CORES)  # per-core slot count (uniform SPMD)
    while len(slots) < n_slots * N_CORES:
        slots.append((0, np.empty(0, dtype=np.int64), np.empty(0, dtype=np.float32)))
    slots.sort(key=lambda t: -len(t[1]))

    # per-rank padded column counts (same on every core => one SPMD program)
    ns_list = []
    for s in range(n_slots):
        group = slots[s * N_CORES : (s + 1) * N_CORES]
        mx = max(len(t[1]) for t in group)
        ns_list.append(min(CAP, max(16, -(-mx // 16) * 16)))
    ns_list = tuple(ns_list)

    if DTYPE == "fp16":
        np_in = np.float16
        prep = lambda a: a.astype(np.float16)
    elif DTYPE == "tf32":
        np_in = np.float32
        prep = _round_tf32
    else:
        np_in = np.float32
        prep = lambda a: a.astype(np.float32)

    key = (ns_list, DTYPE)
    if key not in _nc_cache:
        _nc_cache[key] = _build_nc(ns_list, DTYPE)
    nc = _nc_cache[key]

    w1r = W1.reshape(N_EXP, 2, 128, H_DIM).transpose(0, 2, 1, 3)  # [e,p,cc,h]
    w2r = W2.reshape(N_EXP, 8, 128, C_DIM).transpose(0, 2, 1, 3)  # [e,p,hc,co]
    wpk_all = np.empty((N_EXP, 128, 4096), dtype=np_in)
    wpk_all[:, :, 0:2048] = prep(w1r.reshape(N_EXP, 128, 2048))
    wpk_all[:, :, 2048:4096] = prep(w2r.reshape(N_EXP, 128, 2048))
    xf_in = prep(xf)  # [t, c] in input dtype

    xoff = np.concatenate([[0], np.cumsum([2 * n for n in ns_list])])
    in_maps = []
    for c in range(N_CORES):
        wpk_arr = np.empty((n_slots, 128, 4096), dtype=np_in)
        xtp_arr = np.zeros((128, int(xoff[-1])), dtype=np_in)
        for s in range(n_slots):
            e, toks, _ = slots[s * N_CORES + c]
            wpk_arr[s] = wpk_all[e]
            n = ns_list[s]
            if len(toks):
                xg = xf_in[toks].T.reshape(2, 128, len(toks))  # [cc,p,t]
                base = int(xoff[s])
                xtp_arr[:, base : base + len(toks)] = xg[0]
                xtp_arr[:, base + n : base + n + len(toks)] = xg[1]
        in_maps.append({"wpk": wpk_arr, "xtp": xtp_arr})

    from concourse.bass_utils import run_bass_kernel_spmd

    res = run_bass_kernel_spmd(
        nc,
        in_maps,
        list(range(N_CORES)),
        trace=trace,
        trace_cores=trace_cores,
    )

    out = np.zeros((n_tok, C_DIM), dtype=np.float32)
    for c in range(N_CORES):
        for s in range(n_slots):
            e, toks, wts = slots[s * N_CORES + c]
            if not len(toks):
                continue
            n = ns_list[s]
            yt = res.results[c][f"yt{s}"]  # [128, 2*n]
            # Y^T[oc*128+p, t] = yt[p, oc*n + t]
            y = yt.reshape(128, 2, n).transpose(1, 0, 2).reshape(C_DIM, n)
            out[toks] += wts[:, None] * y[:, 0 : len(toks)].T
    return out.reshape(x.shape).astype(np.float32), ws, res


def kernel(x, Wg, W1, W2):
    out, ws, _ = run(x, Wg, W1, W2, trace=False)
    return out, ws
